# revision 13
# baseline (speedup 1.0000x reference)
"""Trainium2 Bass kernel for segment-wise Conv1d + ReLU + BatchNorm1d.

Reference computation (nn_ConvSeg):
  - x_all [32768, 256] fp32, segment_key [32768] sorted ids (<= 8 segments)
  - per-segment Conv1d (kernel K=9, zero padding 4 at segment boundaries)
  - ReLU, then BatchNorm1d over all tokens (training stats, biased var)

Strategy:
  - Host inserts 4 zero rows at each segment boundary -> the ragged
    per-segment conv becomes ONE dense conv over the gapped sequence.
  - The gapped sequence (8*4104 positions) is split into 8 equal chunks
    (one per NeuronCore) with a 4-position halo on each side.
  - Data is transposed to [d, position] so each conv tap is a shifted
    column window of the same SBUF tile: conv = sum over (tap, d-chunk) of
    128x128 bf16 matmuls accumulated in fp32 PSUM ([d_out-chunk, pos]).
    bf16 inputs keep the conv at the PE's 1 column/cycle peak while
    halving input DMA bytes (measured rel err ~2.7e-3, tolerance 2e-2).
  - A few matmuls on a scratch tile run during the input-DMA head so the
    PE activity monitor un-throttles the clock before the real matmuls.
  - ScalarE fuses bias + ReLU from PSUM and accumulates per-block column
    sums (accum_out); a second ScalarE pass accumulates sums of squares.
    Results DMA out per 2-block group as soon as ready, overlapping the
    remaining matmuls. Raw (unmasked) per-core sums ship as [128, 4].
  - The BatchNorm reduction across cores and the per-channel affine fold
    into the host-side unshard: the host subtracts the gap columns'
    contribution from the raw sums (exact - it has the same f32 y values
    the device summed), reduces across the 8 cores, and applies
    y*scale+shift while reassembling [32768, 256]. No collective, no
    second device pass.
"""

import numpy as np
import ml_dtypes

import concourse.bacc as bacc
import concourse.mybir as mybir
from concourse import tile
from concourse.bass_utils import run_bass_kernel_spmd

F32 = mybir.dt.float32
BF16 = mybir.dt.bfloat16
AF = mybir.ActivationFunctionType
OP = mybir.AluOpType
AX = mybir.AxisListType

N = 32768
D = 256  # d_in == d_out == 256
K = 9
PAD = K // 2
EPS = 1e-5

NCORES = 8
NB = 9  # matmul blocks per core
BS = 456  # positions per block (<= 512 PSUM fp32 bank limit)
L = NB * BS  # 4104 gapped positions per core
LH = L + 2 * PAD  # input columns incl. halo
GAP = 4  # zero rows inserted at each segment boundary (>= PAD)

_PROGRAM_CACHE: dict = {}


def build_program(repeat: int = 1, warm: int = 8):
    """Build + compile the SPMD Bass program (identical on all 8 cores)."""
    nc = bacc.Bacc(
        "TRN2", target_bir_lowering=False, debug=False, num_devices=NCORES
    )

    x_d = nc.declare_dram_parameter("x", [2, 128, LH], BF16, isOutput=False)
    w_d = nc.declare_dram_parameter("w", [2, 128, K * D], BF16, isOutput=False)
    b2_d = nc.declare_dram_parameter("b2", [128, 2], F32, isOutput=False)
    out_d = nc.declare_dram_parameter("out", [D, L], F32, isOutput=True)
    st_d = nc.declare_dram_parameter("st", [128, 4 * NB], F32, isOutput=True)

    with tile.TileContext(nc) as tc:
        with (
            tc.tile_pool(name="const", bufs=1) as const,
            tc.tile_pool(name="ypool", bufs=1) as ypool,
            tc.tile_pool(name="psum", bufs=4, space="PSUM") as psum,
            tc.tile_pool(name="pswarm", bufs=1, space="PSUM") as pswarm,
            tc.tile_pool(name="work", bufs=2) as work,
            tc.tile_pool(name="stats", bufs=1) as stats,
        ):
            xt = [const.tile([128, LH], BF16, tag=f"xt{dc}", name=f"xt{dc}")
                  for dc in range(2)]
            wt = [const.tile([128, K * D], BF16, tag=f"wt{dc}", name=f"wt{dc}")
                  for dc in range(2)]
            b2t = const.tile([128, 2], F32)
            # scratch warmup operand: never written, contents irrelevant
            wz = const.tile([128, BS + 2 * PAD], BF16, tag="wz", name="wz")
            ybig = ypool.tile([128, 2 * NB * BS], F32)
            # per-block raw sums: cols [0,18) = sum(y), [18,36) = sum(y^2)
            stq = stats.tile([128, 4 * NB], F32)

            if warm:
                nc.gpsimd.memset(wz[:], 0.0)

            for _ in range(repeat):
                # --- PE warmup: no data deps, runs during the DMA head so
                # the activity monitor un-throttles the clock ---
                if warm:
                    psw = pswarm.tile([128, BS], F32, tag="psw")
                    for _ in range(warm):
                        nc.tensor.matmul(
                            psw[:], wz[:, 0:128], wz[:, 0:BS],
                            start=True, stop=True,
                        )

                # --- input DMAs, ordered to match PE consumption times ---
                for dc in range(2):  # block-0 x slice
                    nc.sync.dma_start(
                        xt[dc][:, 0 : BS + 2 * PAD], x_d[dc, :, 0 : BS + 2 * PAD]
                    )
                for dc in range(2):  # tap k=0 weights
                    nc.sync.dma_start(wt[dc][:, 0:D], w_d[dc, :, 0:D])
                for dc in range(2):  # remaining weights in one shot
                    nc.sync.dma_start(wt[dc][:, D:], w_d[dc, :, D:])
                for dc in range(2):  # block-1 x slice
                    nc.sync.dma_start(
                        xt[dc][:, BS : 2 * BS + 2 * PAD],
                        x_d[dc, :, BS : 2 * BS + 2 * PAD],
                    )
                nc.sync.dma_start(b2t[:], b2_d[:])  # needed by first relu
                for dc in range(2):  # x blocks 2-4
                    nc.sync.dma_start(
                        xt[dc][:, 2 * BS : 5 * BS + 2 * PAD],
                        x_d[dc, :, 2 * BS : 5 * BS + 2 * PAD],
                    )
                for dc in range(2):  # x blocks 5-8
                    nc.sync.dma_start(
                        xt[dc][:, 5 * BS : LH], x_d[dc, :, 5 * BS : LH]
                    )

                # --- conv + relu(+bias) + raw stats + streaming out-DMA ---
                for b in range(NB):
                    for oc in range(2):
                        ps = psum.tile([128, BS], F32, tag="ps")
                        # dc-major so the dc=0 taps can run while the dc=1
                        # weight DMA is still in flight on the first group
                        for dc in range(2):
                            for k in range(K):
                                nc.tensor.matmul(
                                    ps[:],
                                    wt[dc][
                                        :, k * D + oc * 128 : k * D + oc * 128 + 128
                                    ],
                                    xt[dc][:, b * BS + k : b * BS + k + BS],
                                    start=(k == 0 and dc == 0),
                                    stop=(k == K - 1 and dc == 1),
                                )
                        j = oc * NB + b
                        ysl = ybig[:, j * BS : (j + 1) * BS]
                        # y = relu(conv + bias); accum_out = per-block sum(y)
                        nc.scalar.activation(
                            ysl, ps[:], AF.Relu,
                            bias=b2t[:, oc : oc + 1], scale=1.0,
                            accum_out=stq[:, j : j + 1],
                        )
                        # sum of squares via a second ScalarE pass
                        # (tensor_tensor_reduce crashes the device here)
                        sq = work.tile([128, BS], F32, tag="sq")
                        nc.scalar.activation(
                            sq[:], ysl, AF.Square, bias=0.0, scale=1.0,
                            accum_out=stq[:, 2 * NB + j : 2 * NB + j + 1],
                        )
                    if b % 2 == 1 or b == NB - 1:
                        blo = (b // 2) * 2 if b % 2 == 1 else b
                        ncols = (b - blo + 1) * BS
                        for oc in range(2):
                            nc.sync.dma_start(
                                out_d[oc * 128 : (oc + 1) * 128,
                                      blo * BS : blo * BS + ncols],
                                ybig[:, (oc * NB + blo) * BS
                                     : (oc * NB + blo) * BS + ncols],
                            )

                # --- ship raw per-block stats (host does the tiny reduce) ---
                nc.sync.dma_start(st_d[:], stq[:])

    nc.compile()
    return nc


def _get_program(repeat: int = 1):
    key = repeat
    if key not in _PROGRAM_CACHE:
        _PROGRAM_CACHE[key] = build_program(repeat)
    return _PROGRAM_CACHE[key]


def prepare_inputs(x_all, W, b, gamma, beta, segment_key):
    """Host-side sharding: gap insertion, transpose, per-core slicing.

    Returns (in_maps, aux); aux carries everything assemble_output needs.
    """
    x_all = np.ascontiguousarray(np.asarray(x_all, dtype=np.float32))
    W = np.asarray(W, dtype=np.float32)
    b = np.asarray(b, dtype=np.float32)
    gamma = np.asarray(gamma, dtype=np.float32)
    beta = np.asarray(beta, dtype=np.float32)
    seg = np.asarray(segment_key).reshape(-1)
    n = x_all.shape[0]
    assert n == N, f"kernel hardcodes N={N}, got {n}"

    # run-length segments of the sorted key
    change = np.flatnonzero(seg[1:] != seg[:-1]) + 1
    starts = np.concatenate(([0], change))
    ends = np.concatenate((change, [n]))
    nseg = len(starts)
    assert n + GAP * (nseg + 1) <= NCORES * L, "gapped sequence does not fit"

    # gapped position of each token
    tok_gpos = np.empty(n, dtype=np.int64)
    g = GAP
    for s, e in zip(starts, ends):
        tok_gpos[s:e] = g + np.arange(e - s)
        g += (e - s) + GAP

    # gapped, transposed input with halo: xg_t[:, PAD + gpos] = x_all[n]
    total = NCORES * L
    xg = np.zeros((total + 2 * PAD, D), dtype=np.float32)
    xg[PAD + tok_gpos] = x_all
    xg_t = np.ascontiguousarray(xg.T.astype(ml_dtypes.bfloat16))

    # weights: wmat[d, k*D + o] = W[o, d, k]
    wmat = W.transpose(1, 2, 0).reshape(D, K * D).astype(ml_dtypes.bfloat16)
    w_in = np.ascontiguousarray(wmat.reshape(2, 128, K * D))

    b2 = np.ascontiguousarray(np.stack([b[:128], b[128:]], axis=1))

    in_maps = []
    for c in range(NCORES):
        xc = np.ascontiguousarray(
            xg_t[:, c * L : c * L + LH].reshape(2, 128, LH)
        )
        in_maps.append({"x": xc, "w": w_in, "b2": b2})
    aux = {"tok_gpos": tok_gpos, "gamma": gamma, "beta": beta}
    return in_maps, aux


def assemble_output(results, aux):
    """Unshard + fold the BatchNorm affine.

    Device sums include the gap columns; subtract their contribution (from
    the very same f32 y values the device summed), reduce across cores,
    then apply y*scale + shift per channel while gathering.
    """
    tok_gpos = aux["tok_gpos"]
    gamma, beta = aux["gamma"], aux["beta"]
    core = tok_gpos // L
    loc = tok_gpos % L

    S = np.zeros(D, dtype=np.float64)
    Q = np.zeros(D, dtype=np.float64)
    for c in range(NCORES):
        st = results[c]["st"].astype(np.float64)
        S += np.concatenate(
            [st[:, 0:NB].sum(axis=1), st[:, NB : 2 * NB].sum(axis=1)]
        )
        Q += np.concatenate(
            [st[:, 2 * NB : 3 * NB].sum(axis=1), st[:, 3 * NB :].sum(axis=1)]
        )
    valid = np.zeros((NCORES, L), dtype=bool)
    valid[core, loc] = True
    for c in range(NCORES):
        yg = results[c]["out"][:, ~valid[c]].astype(np.float64)  # [256, ngap]
        S -= yg.sum(axis=1)
        Q -= (yg * yg).sum(axis=1)

    mean = S / N
    var = Q / N - mean * mean
    scale = gamma.astype(np.float64) / np.sqrt(var + EPS)
    shift = beta.astype(np.float64) - mean * scale
    scale32 = scale.astype(np.float32)
    shift32 = shift.astype(np.float32)

    out = np.empty((N, D), dtype=np.float32)
    for c in range(NCORES):
        sel = core == c
        out[sel] = results[c]["out"][:, loc[sel]].T * scale32 + shift32
    return out


def kernel(x_all, W, b, gamma, beta, segment_key):
    nc = _get_program()
    in_maps, aux = prepare_inputs(x_all, W, b, gamma, beta, segment_key)
    res = run_bass_kernel_spmd(nc, in_maps, list(range(NCORES)))
    return assemble_output(res.results, aux)


# revision 17
# speedup vs baseline: 2.8424x; 2.8424x over previous
"""Trainium2 Bass kernel for segment-wise Conv1d + ReLU + BatchNorm1d.

Reference computation (nn_ConvSeg):
  - x_all [32768, 256] fp32, segment_key [32768] sorted ids (<= 8 segments)
  - per-segment Conv1d (kernel K=9, zero padding 4 at segment boundaries)
  - ReLU, then BatchNorm1d over all tokens (training stats, biased var)

Strategy:
  - Host inserts 4 zero rows at each segment boundary -> the ragged
    per-segment conv becomes ONE dense conv over the gapped sequence.
  - The gapped sequence (8*4104 positions) is split into 8 equal chunks
    (one per NeuronCore) with a 4-position halo on each side.
  - Data is transposed to [d, position] so each conv tap is a shifted
    column window of the same SBUF tile: conv = sum over (tap, d-chunk) of
    128x128 bf16 matmuls accumulated in fp32 PSUM ([d_out-chunk, pos]).
    bf16 inputs keep the conv at the PE's 1 column/cycle peak while
    halving input DMA bytes (measured rel err ~2.7e-3, tolerance 2e-2).
  - A few matmuls on a scratch tile run during the input-DMA head so the
    PE activity monitor un-throttles the clock before the real matmuls.
  - ScalarE fuses bias + ReLU from PSUM and accumulates per-block column
    sums (accum_out); a second ScalarE pass accumulates sums of squares.
    Results DMA out per 2-block group as soon as ready, overlapping the
    remaining matmuls. Raw (unmasked) per-core sums ship as [128, 4].
  - The BatchNorm reduction across cores and the per-channel affine fold
    into the host-side unshard: the host subtracts the gap columns'
    contribution from the raw sums (exact - it has the same f32 y values
    the device summed), reduces across the 8 cores, and applies
    y*scale+shift while reassembling [32768, 256]. No collective, no
    second device pass.
"""

import numpy as np
import ml_dtypes

import concourse.bacc as bacc
import concourse.mybir as mybir
from concourse import tile
from concourse.bass_utils import run_bass_kernel_spmd

F32 = mybir.dt.float32
BF16 = mybir.dt.bfloat16
AF = mybir.ActivationFunctionType
OP = mybir.AluOpType
AX = mybir.AxisListType

N = 32768
D = 256  # d_in == d_out == 256
K = 9
PAD = K // 2
EPS = 1e-5

NCORES = 8
NB = 18  # matmul blocks per core
BS = 228  # positions per block (lives in the PE's fast free-dim regime)
L = NB * BS  # 4104 gapped positions per core
LH = L + 2 * PAD  # input columns incl. halo
GAP = 4  # zero rows inserted at each segment boundary (>= PAD)

# out-DMA column groups and x-DMA chunks (all boundaries are multiples of
# every supported block size)
OUTG = [(0, 912), (912, 1824), (1824, 2736), (2736, 3648), (3648, L)]
XCH = [(0, 464), (456, 920), (912, 2288), (2280, LH)]

_PROGRAM_CACHE: dict = {}


def build_program(repeat: int = 1, warm: int = 8, nb: int = None,
                  bs: int = None):
    """Build + compile the SPMD Bass program (identical on all 8 cores)."""
    nb = NB if nb is None else nb
    bs = BS if bs is None else bs
    assert nb * bs == L
    nc = bacc.Bacc(
        "TRN2", target_bir_lowering=False, debug=False, num_devices=NCORES
    )

    x_d = nc.declare_dram_parameter("x", [2, 128, LH], BF16, isOutput=False)
    w_d = nc.declare_dram_parameter("w", [2, 128, K * D], BF16, isOutput=False)
    b2_d = nc.declare_dram_parameter("b2", [128, 2], F32, isOutput=False)
    out_d = nc.declare_dram_parameter("out", [D, L], F32, isOutput=True)
    st_d = nc.declare_dram_parameter("st", [128, 4 * nb], F32, isOutput=True)

    with tile.TileContext(nc) as tc:
        with (
            tc.tile_pool(name="const", bufs=1) as const,
            tc.tile_pool(name="ypool", bufs=1) as ypool,
            tc.tile_pool(name="psum", bufs=4, space="PSUM") as psum,
            tc.tile_pool(name="pswarm", bufs=1, space="PSUM") as pswarm,
            tc.tile_pool(name="work", bufs=2) as work,
            tc.tile_pool(name="stats", bufs=1) as stats,
        ):
            xt = [const.tile([128, LH], BF16, tag=f"xt{dc}", name=f"xt{dc}")
                  for dc in range(2)]
            wt = [const.tile([128, K * D], BF16, tag=f"wt{dc}", name=f"wt{dc}")
                  for dc in range(2)]
            b2t = const.tile([128, 2], F32)
            # scratch warmup operand: never written, contents irrelevant
            wz = const.tile([128, 464], BF16, tag="wz", name="wz")
            ybig = ypool.tile([128, 2 * L], F32)
            # per-block raw sums: cols [0,2nb) = sum(y), [2nb,4nb) = sum(y^2)
            stq = stats.tile([128, 4 * nb], F32)

            if warm:
                nc.gpsimd.memset(wz[:], 0.0)

            for _ in range(repeat):
                # --- PE warmup: no data deps, runs during the DMA head so
                # the activity monitor un-throttles the clock ---
                if warm:
                    psw = pswarm.tile([128, min(bs, 464)], F32, tag="psw")
                    for _ in range(warm):
                        nc.tensor.matmul(
                            psw[:], wz[:, 0:128], wz[:, 0 : min(bs, 464)],
                            start=True, stop=True,
                        )

                # --- input DMAs, ordered to match PE consumption times ---
                for dc in range(2):  # first x chunk
                    lo, hi = XCH[0]
                    nc.sync.dma_start(xt[dc][:, lo:hi], x_d[dc, :, lo:hi])
                for dc in range(2):  # tap k=0 weights
                    nc.sync.dma_start(wt[dc][:, 0:D], w_d[dc, :, 0:D])
                for dc in range(2):  # remaining weights in one shot
                    nc.sync.dma_start(wt[dc][:, D:], w_d[dc, :, D:])
                for dc in range(2):  # second x chunk
                    lo, hi = XCH[1]
                    nc.sync.dma_start(xt[dc][:, lo:hi], x_d[dc, :, lo:hi])
                nc.sync.dma_start(b2t[:], b2_d[:])  # needed by first relu
                for ch in range(2, len(XCH)):
                    lo, hi = XCH[ch]
                    for dc in range(2):
                        nc.sync.dma_start(xt[dc][:, lo:hi], x_d[dc, :, lo:hi])

                # --- conv + relu(+bias) + raw stats + streaming out-DMA ---
                for glo, ghi in OUTG:
                    for b in range(glo // bs, ghi // bs):
                        for oc in range(2):
                            ps = psum.tile([128, bs], F32, tag="ps")
                            # dc-major so the dc=0 taps can run while the
                            # dc=1 weight DMA is still in flight early on
                            for dc in range(2):
                                for k in range(K):
                                    nc.tensor.matmul(
                                        ps[:],
                                        wt[dc][
                                            :, k * D + oc * 128
                                            : k * D + oc * 128 + 128
                                        ],
                                        xt[dc][:, b * bs + k : b * bs + k + bs],
                                        start=(k == 0 and dc == 0),
                                        stop=(k == K - 1 and dc == 1),
                                    )
                            j = oc * nb + b
                            ysl = ybig[:, oc * L + b * bs : oc * L + (b + 1) * bs]
                            # y = relu(conv + bias); accum_out = sum(y)
                            nc.scalar.activation(
                                ysl, ps[:], AF.Relu,
                                bias=b2t[:, oc : oc + 1], scale=1.0,
                                accum_out=stq[:, j : j + 1],
                            )
                            # sum of squares via a second ScalarE pass
                            # (tensor_tensor_reduce crashes the device here)
                            sq = work.tile([128, bs], F32, tag="sq")
                            nc.scalar.activation(
                                sq[:], ysl, AF.Square, bias=0.0, scale=1.0,
                                accum_out=stq[:, 2 * nb + j : 2 * nb + j + 1],
                            )
                    for oc in range(2):
                        nc.sync.dma_start(
                            out_d[oc * 128 : (oc + 1) * 128, glo:ghi],
                            ybig[:, oc * L + glo : oc * L + ghi],
                        )

                # --- ship raw per-block stats (host does the tiny reduce) ---
                nc.sync.dma_start(st_d[:], stq[:])

    nc.compile()
    return nc


def _get_program(repeat: int = 1):
    key = repeat
    if key not in _PROGRAM_CACHE:
        _PROGRAM_CACHE[key] = build_program(repeat)
    return _PROGRAM_CACHE[key]


def prepare_inputs(x_all, W, b, gamma, beta, segment_key):
    """Host-side sharding: gap insertion, transpose, per-core slicing.

    Returns (in_maps, aux); aux carries everything assemble_output needs.
    """
    x_all = np.ascontiguousarray(np.asarray(x_all, dtype=np.float32))
    W = np.asarray(W, dtype=np.float32)
    b = np.asarray(b, dtype=np.float32)
    gamma = np.asarray(gamma, dtype=np.float32)
    beta = np.asarray(beta, dtype=np.float32)
    seg = np.asarray(segment_key).reshape(-1)
    n = x_all.shape[0]
    assert n == N, f"kernel hardcodes N={N}, got {n}"

    # run-length segments of the sorted key
    change = np.flatnonzero(seg[1:] != seg[:-1]) + 1
    starts = np.concatenate(([0], change))
    ends = np.concatenate((change, [n]))
    nseg = len(starts)
    assert n + GAP * (nseg + 1) <= NCORES * L, "gapped sequence does not fit"

    # gapped position of each token
    tok_gpos = np.empty(n, dtype=np.int64)
    g = GAP
    for s, e in zip(starts, ends):
        tok_gpos[s:e] = g + np.arange(e - s)
        g += (e - s) + GAP

    # gapped, transposed input with halo: xg_t[:, PAD + gpos] = x_all[n]
    total = NCORES * L
    xg = np.zeros((total + 2 * PAD, D), dtype=np.float32)
    xg[PAD + tok_gpos] = x_all
    xg_t = np.ascontiguousarray(xg.T.astype(ml_dtypes.bfloat16))

    # weights: wmat[d, k*D + o] = W[o, d, k]
    wmat = W.transpose(1, 2, 0).reshape(D, K * D).astype(ml_dtypes.bfloat16)
    w_in = np.ascontiguousarray(wmat.reshape(2, 128, K * D))

    b2 = np.ascontiguousarray(np.stack([b[:128], b[128:]], axis=1))

    in_maps = []
    for c in range(NCORES):
        xc = np.ascontiguousarray(
            xg_t[:, c * L : c * L + LH].reshape(2, 128, LH)
        )
        in_maps.append({"x": xc, "w": w_in, "b2": b2})
    aux = {"tok_gpos": tok_gpos, "gamma": gamma, "beta": beta}
    return in_maps, aux


def assemble_output(results, aux):
    """Unshard + fold the BatchNorm affine.

    Device sums include the gap columns; subtract their contribution (from
    the very same f32 y values the device summed), reduce across cores,
    then apply y*scale + shift per channel while gathering.
    """
    tok_gpos = aux["tok_gpos"]
    gamma, beta = aux["gamma"], aux["beta"]
    core = tok_gpos // L
    loc = tok_gpos % L

    S = np.zeros(D, dtype=np.float64)
    Q = np.zeros(D, dtype=np.float64)
    for c in range(NCORES):
        st = results[c]["st"].astype(np.float64)
        S += np.concatenate(
            [st[:, 0:NB].sum(axis=1), st[:, NB : 2 * NB].sum(axis=1)]
        )
        Q += np.concatenate(
            [st[:, 2 * NB : 3 * NB].sum(axis=1), st[:, 3 * NB :].sum(axis=1)]
        )
    valid = np.zeros((NCORES, L), dtype=bool)
    valid[core, loc] = True
    for c in range(NCORES):
        yg = results[c]["out"][:, ~valid[c]].astype(np.float64)  # [256, ngap]
        S -= yg.sum(axis=1)
        Q -= (yg * yg).sum(axis=1)

    mean = S / N
    var = Q / N - mean * mean
    scale = gamma.astype(np.float64) / np.sqrt(var + EPS)
    shift = beta.astype(np.float64) - mean * scale
    scale32 = scale.astype(np.float32)
    shift32 = shift.astype(np.float32)

    out = np.empty((N, D), dtype=np.float32)
    for c in range(NCORES):
        sel = core == c
        out[sel] = results[c]["out"][:, loc[sel]].T * scale32 + shift32
    return out


def kernel(x_all, W, b, gamma, beta, segment_key):
    nc = _get_program()
    in_maps, aux = prepare_inputs(x_all, W, b, gamma, beta, segment_key)
    res = run_bass_kernel_spmd(nc, in_maps, list(range(NCORES)))
    return assemble_output(res.results, aux)


# revision 21
# speedup vs baseline: 4.3444x; 1.5284x over previous
"""Trainium2 Bass kernel for segment-wise Conv1d + ReLU + BatchNorm1d.

Reference computation (nn_ConvSeg):
  - x_all [32768, 256] fp32, segment_key [32768] sorted ids (<= 8 segments)
  - per-segment Conv1d (kernel K=9, zero padding 4 at segment boundaries)
  - ReLU, then BatchNorm1d over all tokens (training stats, biased var)

Strategy:
  - Host inserts 4 zero rows at each segment boundary -> the ragged
    per-segment conv becomes ONE dense conv over the gapped sequence.
  - The gapped sequence (8*4104 positions) is split into 8 equal chunks
    (one per NeuronCore) with a 4-position halo on each side.
  - Data is transposed to [d, position] so each conv tap is a shifted
    column window of the same SBUF tile: conv = sum over (tap, d-chunk) of
    128x128 bf16 matmuls accumulated in fp32 PSUM ([d_out-chunk, pos]).
    bf16 inputs keep the conv at the PE's 1 column/cycle peak while
    halving input DMA bytes (measured rel err ~2.7e-3, tolerance 2e-2).
  - A few matmuls on a scratch tile run during the input-DMA head so the
    PE activity monitor un-throttles the clock before the real matmuls.
  - ScalarE fuses bias + ReLU from PSUM and accumulates per-block column
    sums (accum_out); a second ScalarE pass accumulates sums of squares.
    Results DMA out per 2-block group as soon as ready, overlapping the
    remaining matmuls. Raw (unmasked) per-core sums ship as [128, 4].
  - The BatchNorm reduction across cores and the per-channel affine fold
    into the host-side unshard: the host subtracts the gap columns'
    contribution from the raw sums (exact - it has the same f32 y values
    the device summed), reduces across the 8 cores, and applies
    y*scale+shift while reassembling [32768, 256]. No collective, no
    second device pass.
"""

import numpy as np
import ml_dtypes

import concourse.bacc as bacc
import concourse.mybir as mybir
from concourse import tile
from concourse.bass_utils import run_bass_kernel_spmd

F32 = mybir.dt.float32
BF16 = mybir.dt.bfloat16
AF = mybir.ActivationFunctionType
OP = mybir.AluOpType
AX = mybir.AxisListType

N = 32768
D = 256  # d_in == d_out == 256
K = 9
PAD = K // 2
EPS = 1e-5

NCORES = 8
NB = 18  # matmul blocks per core
BS = 228  # positions per block (lives in the PE's fast free-dim regime)
L = NB * BS  # 4104 gapped positions per core
LH = L + 2 * PAD  # input columns incl. halo
GAP = 4  # zero rows inserted at each segment boundary (>= PAD)

# out-DMA column groups and x-DMA chunks (all boundaries are multiples of
# every supported block size)
OUTG = [(0, 912), (912, 1824), (1824, 2736), (2736, 3648), (3648, L)]
XCH = [(0, 464), (456, 920), (912, 2288), (2280, LH)]

_PROGRAM_CACHE: dict = {}


def build_program(repeat: int = 1, warm: int = 8, nb: int = None,
                  bs: int = None, out_bf16: bool = True):
    """Build + compile the SPMD Bass program (identical on all 8 cores)."""
    nb = NB if nb is None else nb
    bs = BS if bs is None else bs
    assert nb * bs == L
    ydt = BF16 if out_bf16 else F32
    nc = bacc.Bacc(
        "TRN2", target_bir_lowering=False, debug=False, num_devices=NCORES
    )

    x_d = nc.declare_dram_parameter("x", [2, 128, LH], BF16, isOutput=False)
    w_d = nc.declare_dram_parameter("w", [2, 128, K * D], BF16, isOutput=False)
    b2_d = nc.declare_dram_parameter("b2", [128, 2], F32, isOutput=False)
    out_d = nc.declare_dram_parameter("out", [D, L], ydt, isOutput=True)
    st_d = nc.declare_dram_parameter("st", [128, 4 * nb], F32, isOutput=True)

    with tile.TileContext(nc) as tc:
        with (
            tc.tile_pool(name="const", bufs=1) as const,
            tc.tile_pool(name="ypool", bufs=1) as ypool,
            tc.tile_pool(name="psum", bufs=4, space="PSUM") as psum,
            tc.tile_pool(name="pswarm", bufs=1, space="PSUM") as pswarm,
            tc.tile_pool(name="work", bufs=2) as work,
            tc.tile_pool(name="stats", bufs=1) as stats,
        ):
            xt = [const.tile([128, LH], BF16, tag=f"xt{dc}", name=f"xt{dc}")
                  for dc in range(2)]
            wt = [const.tile([128, K * D], BF16, tag=f"wt{dc}", name=f"wt{dc}")
                  for dc in range(2)]
            b2t = const.tile([128, 2], F32)
            # scratch warmup operand: never written, contents irrelevant
            wz = const.tile([128, 464], BF16, tag="wz", name="wz")
            ybig = ypool.tile([128, 2 * L], ydt)
            # per-block raw sums: cols [0,2nb) = sum(y), [2nb,4nb) = sum(y^2)
            stq = stats.tile([128, 4 * nb], F32)

            if warm:
                nc.gpsimd.memset(wz[:], 0.0)

            for _ in range(repeat):
                # --- PE warmup: no data deps, runs during the DMA head so
                # the activity monitor un-throttles the clock ---
                if warm:
                    psw = pswarm.tile([128, min(bs, 464)], F32, tag="psw")
                    for _ in range(warm):
                        nc.tensor.matmul(
                            psw[:], wz[:, 0:128], wz[:, 0 : min(bs, 464)],
                            start=True, stop=True,
                        )

                # --- input DMAs, ordered to match PE consumption times ---
                for dc in range(2):  # first x chunk
                    lo, hi = XCH[0]
                    nc.sync.dma_start(xt[dc][:, lo:hi], x_d[dc, :, lo:hi])
                for dc in range(2):  # tap k=0 weights
                    nc.sync.dma_start(wt[dc][:, 0:D], w_d[dc, :, 0:D])
                for dc in range(2):  # remaining weights in one shot
                    nc.sync.dma_start(wt[dc][:, D:], w_d[dc, :, D:])
                for dc in range(2):  # second x chunk
                    lo, hi = XCH[1]
                    nc.sync.dma_start(xt[dc][:, lo:hi], x_d[dc, :, lo:hi])
                nc.sync.dma_start(b2t[:], b2_d[:])  # needed by first relu
                for ch in range(2, len(XCH)):
                    lo, hi = XCH[ch]
                    for dc in range(2):
                        nc.sync.dma_start(xt[dc][:, lo:hi], x_d[dc, :, lo:hi])

                # --- conv + relu(+bias) + raw stats + streaming out-DMA ---
                for glo, ghi in OUTG:
                    for b in range(glo // bs, ghi // bs):
                        for oc in range(2):
                            ps = psum.tile([128, bs], F32, tag="ps")
                            # dc-major so the dc=0 taps can run while the
                            # dc=1 weight DMA is still in flight early on
                            for dc in range(2):
                                for k in range(K):
                                    nc.tensor.matmul(
                                        ps[:],
                                        wt[dc][
                                            :, k * D + oc * 128
                                            : k * D + oc * 128 + 128
                                        ],
                                        xt[dc][:, b * bs + k : b * bs + k + bs],
                                        start=(k == 0 and dc == 0),
                                        stop=(k == K - 1 and dc == 1),
                                    )
                            j = oc * nb + b
                            ysl = ybig[:, oc * L + b * bs : oc * L + (b + 1) * bs]
                            # y = relu(conv + bias); accum_out = sum(y)
                            nc.scalar.activation(
                                ysl, ps[:], AF.Relu,
                                bias=b2t[:, oc : oc + 1], scale=1.0,
                                accum_out=stq[:, j : j + 1],
                            )
                            # sum of squares via a second ScalarE pass
                            # (tensor_tensor_reduce crashes the device here)
                            sq = work.tile([128, bs], F32, tag="sq")
                            nc.scalar.activation(
                                sq[:], ysl, AF.Square, bias=0.0, scale=1.0,
                                accum_out=stq[:, 2 * nb + j : 2 * nb + j + 1],
                            )
                    for oc in range(2):
                        nc.sync.dma_start(
                            out_d[oc * 128 : (oc + 1) * 128, glo:ghi],
                            ybig[:, oc * L + glo : oc * L + ghi],
                        )

                # --- ship raw per-block stats (host does the tiny reduce) ---
                nc.sync.dma_start(st_d[:], stq[:])

    nc.compile()
    return nc


def _get_program(repeat: int = 1):
    key = repeat
    if key not in _PROGRAM_CACHE:
        _PROGRAM_CACHE[key] = build_program(repeat)
    return _PROGRAM_CACHE[key]


def prepare_inputs(x_all, W, b, gamma, beta, segment_key):
    """Host-side sharding: gap insertion, transpose, per-core slicing.

    Returns (in_maps, aux); aux carries everything assemble_output needs.
    """
    x_all = np.ascontiguousarray(np.asarray(x_all, dtype=np.float32))
    W = np.asarray(W, dtype=np.float32)
    b = np.asarray(b, dtype=np.float32)
    gamma = np.asarray(gamma, dtype=np.float32)
    beta = np.asarray(beta, dtype=np.float32)
    seg = np.asarray(segment_key).reshape(-1)
    n = x_all.shape[0]
    assert n == N, f"kernel hardcodes N={N}, got {n}"

    # run-length segments of the sorted key
    change = np.flatnonzero(seg[1:] != seg[:-1]) + 1
    starts = np.concatenate(([0], change))
    ends = np.concatenate((change, [n]))
    nseg = len(starts)
    assert n + GAP * (nseg + 1) <= NCORES * L, "gapped sequence does not fit"

    # gapped position of each token
    tok_gpos = np.empty(n, dtype=np.int64)
    g = GAP
    for s, e in zip(starts, ends):
        tok_gpos[s:e] = g + np.arange(e - s)
        g += (e - s) + GAP

    # gapped, transposed input with halo: xg_t[:, PAD + gpos] = x_all[n]
    total = NCORES * L
    xg = np.zeros((total + 2 * PAD, D), dtype=np.float32)
    xg[PAD + tok_gpos] = x_all
    xg_t = np.ascontiguousarray(xg.T.astype(ml_dtypes.bfloat16))

    # weights: wmat[d, k*D + o] = W[o, d, k]
    wmat = W.transpose(1, 2, 0).reshape(D, K * D).astype(ml_dtypes.bfloat16)
    w_in = np.ascontiguousarray(wmat.reshape(2, 128, K * D))

    b2 = np.ascontiguousarray(np.stack([b[:128], b[128:]], axis=1))

    in_maps = []
    for c in range(NCORES):
        xc = np.ascontiguousarray(
            xg_t[:, c * L : c * L + LH].reshape(2, 128, LH)
        )
        in_maps.append({"x": xc, "w": w_in, "b2": b2})
    aux = {"tok_gpos": tok_gpos, "gamma": gamma, "beta": beta}
    return in_maps, aux


def assemble_output(results, aux):
    """Unshard + fold the BatchNorm affine.

    Device sums include the gap columns; subtract their contribution (from
    the very same f32 y values the device summed), reduce across cores,
    then apply y*scale + shift per channel while gathering.
    """
    tok_gpos = aux["tok_gpos"]
    gamma, beta = aux["gamma"], aux["beta"]
    core = tok_gpos // L
    loc = tok_gpos % L

    S = np.zeros(D, dtype=np.float64)
    Q = np.zeros(D, dtype=np.float64)
    for c in range(NCORES):
        st = results[c]["st"].astype(np.float64)
        S += np.concatenate(
            [st[:, 0:NB].sum(axis=1), st[:, NB : 2 * NB].sum(axis=1)]
        )
        Q += np.concatenate(
            [st[:, 2 * NB : 3 * NB].sum(axis=1), st[:, 3 * NB :].sum(axis=1)]
        )
    valid = np.zeros((NCORES, L), dtype=bool)
    valid[core, loc] = True
    for c in range(NCORES):
        yg = results[c]["out"][:, ~valid[c]].astype(np.float64)  # [256, ngap]
        S -= yg.sum(axis=1)
        Q -= (yg * yg).sum(axis=1)

    mean = S / N
    var = Q / N - mean * mean
    scale = gamma.astype(np.float64) / np.sqrt(var + EPS)
    shift = beta.astype(np.float64) - mean * scale
    scale32 = scale.astype(np.float32)
    shift32 = shift.astype(np.float32)

    out = np.empty((N, D), dtype=np.float32)
    for c in range(NCORES):
        sel = core == c
        yc = results[c]["out"][:, loc[sel]].T.astype(np.float32)
        out[sel] = yc * scale32 + shift32
    return out


def kernel(x_all, W, b, gamma, beta, segment_key):
    nc = _get_program()
    in_maps, aux = prepare_inputs(x_all, W, b, gamma, beta, segment_key)
    res = run_bass_kernel_spmd(nc, in_maps, list(range(NCORES)))
    return assemble_output(res.results, aux)


# revision 23
# speedup vs baseline: 5.9877x; 1.3783x over previous
"""Trainium2 Bass kernel for segment-wise Conv1d + ReLU + BatchNorm1d.

Reference computation (nn_ConvSeg):
  - x_all [32768, 256] fp32, segment_key [32768] sorted ids (<= 8 segments)
  - per-segment Conv1d (kernel K=9, zero padding 4 at segment boundaries)
  - ReLU, then BatchNorm1d over all tokens (training stats, biased var)

Strategy:
  - Host inserts 4 zero rows at each segment boundary -> the ragged
    per-segment conv becomes ONE dense conv over the gapped sequence.
  - The gapped sequence (8*4104 positions) is split into 8 equal chunks
    (one per NeuronCore) with a 4-position halo on each side.
  - Data is transposed to [d, position] so each conv tap is a shifted
    column window of the same SBUF tile: conv = sum over (tap, d-chunk) of
    128x128 bf16 matmuls accumulated in fp32 PSUM ([d_out-chunk, pos]).
    bf16 inputs halve input DMA bytes at ~2.7e-3 rel err (tolerance 2e-2).
  - Blocks are 228 positions: measured on this hardware, matmuls with
    free dim <= ~228 stream ~2.6 cols/ns vs ~1.2 above ~256 - a 2x cliff
    (measured via pure-matmul count-differencing probes).
  - A few matmuls on a scratch tile run during the input-DMA head so the
    PE activity monitor un-throttles the clock before the real matmuls.
  - ScalarE fuses bias + ReLU from PSUM and accumulates per-block column
    sums (accum_out); a second ScalarE pass accumulates sums of squares.
    Results DMA out (bf16) per ~900-column group as soon as ready,
    overlapping the remaining matmuls; raw per-block sums ship last as a
    tiny [128, 4*NB] tensor.
  - The BatchNorm reduction across cores and the per-channel affine fold
    into the host-side unshard: the host subtracts the gap columns'
    contribution from the raw sums, reduces across the 8 cores, and
    applies y*scale+shift while reassembling [32768, 256]. No collective
    (the emulated-NRT AllReduce costs ~1 ms here), no second device pass.
"""

import numpy as np
import ml_dtypes

import concourse.bacc as bacc
import concourse.mybir as mybir
from concourse import tile
from concourse.bass_utils import run_bass_kernel_spmd

F32 = mybir.dt.float32
BF16 = mybir.dt.bfloat16
AF = mybir.ActivationFunctionType

N = 32768
D = 256  # d_in == d_out == 256
K = 9
PAD = K // 2
EPS = 1e-5

NCORES = 8
NB = 18  # matmul blocks per core
BS = 228  # positions per block (lives in the PE's fast free-dim regime)
L = NB * BS  # 4104 gapped positions per core
LH = L + 2 * PAD  # input columns incl. halo
GAP = 4  # zero rows inserted at each segment boundary (>= PAD)

# out-DMA column groups and x-DMA chunks (all boundaries are multiples of
# every supported block size)
OUTG = [(0, 912), (912, 1824), (1824, 2736), (2736, 3648), (3648, L)]
XCH = [(0, 464), (456, 920), (912, 2288), (2280, LH)]

_PROGRAM_CACHE: dict = {}


def build_program(repeat: int = 1, warm: int = 8, nb: int = None,
                  bs: int = None, out_bf16: bool = True):
    """Build + compile the SPMD Bass program (identical on all 8 cores)."""
    nb = NB if nb is None else nb
    bs = BS if bs is None else bs
    assert nb * bs == L
    ydt = BF16 if out_bf16 else F32
    nc = bacc.Bacc(
        "TRN2", target_bir_lowering=False, debug=False, num_devices=NCORES
    )

    x_d = nc.declare_dram_parameter("x", [2, 128, LH], BF16, isOutput=False)
    w_d = nc.declare_dram_parameter("w", [2, 128, K * D], BF16, isOutput=False)
    b2_d = nc.declare_dram_parameter("b2", [128, 2], F32, isOutput=False)
    out_d = nc.declare_dram_parameter("out", [D, L], ydt, isOutput=True)
    st_d = nc.declare_dram_parameter("st", [128, 4 * nb], F32, isOutput=True)

    with tile.TileContext(nc) as tc:
        with (
            tc.tile_pool(name="const", bufs=1) as const,
            tc.tile_pool(name="ypool", bufs=1) as ypool,
            tc.tile_pool(name="psum", bufs=4, space="PSUM") as psum,
            tc.tile_pool(name="pswarm", bufs=1, space="PSUM") as pswarm,
            tc.tile_pool(name="work", bufs=2) as work,
            tc.tile_pool(name="stats", bufs=1) as stats,
        ):
            xt = [const.tile([128, LH], BF16, tag=f"xt{dc}", name=f"xt{dc}")
                  for dc in range(2)]
            wt = [const.tile([128, K * D], BF16, tag=f"wt{dc}", name=f"wt{dc}")
                  for dc in range(2)]
            b2t = const.tile([128, 2], F32)
            # scratch warmup operand: never written, contents irrelevant
            wz = const.tile([128, 464], BF16, tag="wz", name="wz")
            ybig = ypool.tile([128, 2 * L], ydt)
            # per-block raw sums: cols [0,2nb) = sum(y), [2nb,4nb) = sum(y^2)
            stq = stats.tile([128, 4 * nb], F32)

            if warm:
                nc.gpsimd.memset(wz[:], 0.0)

            for _ in range(repeat):
                # --- PE warmup: no data deps, runs during the DMA head so
                # the activity monitor un-throttles the clock ---
                if warm:
                    psw = pswarm.tile([128, min(bs, 464)], F32, tag="psw")
                    for _ in range(warm):
                        nc.tensor.matmul(
                            psw[:], wz[:, 0:128], wz[:, 0 : min(bs, 464)],
                            start=True, stop=True,
                        )

                # --- input DMAs, ordered to match PE consumption times ---
                for dc in range(2):  # first x chunk
                    lo, hi = XCH[0]
                    nc.sync.dma_start(xt[dc][:, lo:hi], x_d[dc, :, lo:hi])
                for dc in range(2):  # tap k=0 weights
                    nc.sync.dma_start(wt[dc][:, 0:D], w_d[dc, :, 0:D])
                for dc in range(2):  # remaining weights in one shot
                    nc.sync.dma_start(wt[dc][:, D:], w_d[dc, :, D:])
                for dc in range(2):  # second x chunk
                    lo, hi = XCH[1]
                    nc.sync.dma_start(xt[dc][:, lo:hi], x_d[dc, :, lo:hi])
                nc.sync.dma_start(b2t[:], b2_d[:])  # needed by first relu
                for ch in range(2, len(XCH)):
                    lo, hi = XCH[ch]
                    for dc in range(2):
                        nc.sync.dma_start(xt[dc][:, lo:hi], x_d[dc, :, lo:hi])

                # --- conv + relu(+bias) + raw stats + streaming out-DMA ---
                for glo, ghi in OUTG:
                    for b in range(glo // bs, ghi // bs):
                        for oc in range(2):
                            ps = psum.tile([128, bs], F32, tag="ps")
                            # dc-major so the dc=0 taps can run while the
                            # dc=1 weight DMA is still in flight early on
                            for dc in range(2):
                                for k in range(K):
                                    nc.tensor.matmul(
                                        ps[:],
                                        wt[dc][
                                            :, k * D + oc * 128
                                            : k * D + oc * 128 + 128
                                        ],
                                        xt[dc][:, b * bs + k : b * bs + k + bs],
                                        start=(k == 0 and dc == 0),
                                        stop=(k == K - 1 and dc == 1),
                                    )
                            j = oc * nb + b
                            ysl = ybig[:, oc * L + b * bs : oc * L + (b + 1) * bs]
                            # y = relu(conv + bias); accum_out = sum(y)
                            nc.scalar.activation(
                                ysl, ps[:], AF.Relu,
                                bias=b2t[:, oc : oc + 1], scale=1.0,
                                accum_out=stq[:, j : j + 1],
                            )
                            # sum of squares via a second ScalarE pass
                            # (tensor_tensor_reduce crashes the device here)
                            sq = work.tile([128, bs], F32, tag="sq")
                            nc.scalar.activation(
                                sq[:], ysl, AF.Square, bias=0.0, scale=1.0,
                                accum_out=stq[:, 2 * nb + j : 2 * nb + j + 1],
                            )
                    for oc in range(2):
                        nc.sync.dma_start(
                            out_d[oc * 128 : (oc + 1) * 128, glo:ghi],
                            ybig[:, oc * L + glo : oc * L + ghi],
                        )

                # --- ship raw per-block stats (host does the tiny reduce) ---
                nc.sync.dma_start(st_d[:], stq[:])

    nc.compile()
    return nc


def _get_program(repeat: int = 1):
    key = repeat
    if key not in _PROGRAM_CACHE:
        _PROGRAM_CACHE[key] = build_program(repeat)
    return _PROGRAM_CACHE[key]


def prepare_inputs(x_all, W, b, gamma, beta, segment_key):
    """Host-side sharding: gap insertion, transpose, per-core slicing.

    Returns (in_maps, aux); aux carries everything assemble_output needs.
    """
    x_all = np.ascontiguousarray(np.asarray(x_all, dtype=np.float32))
    W = np.asarray(W, dtype=np.float32)
    b = np.asarray(b, dtype=np.float32)
    gamma = np.asarray(gamma, dtype=np.float32)
    beta = np.asarray(beta, dtype=np.float32)
    seg = np.asarray(segment_key).reshape(-1)
    n = x_all.shape[0]
    assert n == N, f"kernel hardcodes N={N}, got {n}"

    # run-length segments of the sorted key
    change = np.flatnonzero(seg[1:] != seg[:-1]) + 1
    starts = np.concatenate(([0], change))
    ends = np.concatenate((change, [n]))
    nseg = len(starts)
    assert n + GAP * (nseg + 1) <= NCORES * L, "gapped sequence does not fit"

    # gapped position of each token
    tok_gpos = np.empty(n, dtype=np.int64)
    g = GAP
    for s, e in zip(starts, ends):
        tok_gpos[s:e] = g + np.arange(e - s)
        g += (e - s) + GAP

    # gapped, transposed input with halo: xg_t[:, PAD + gpos] = x_all[n]
    total = NCORES * L
    xg = np.zeros((total + 2 * PAD, D), dtype=np.float32)
    xg[PAD + tok_gpos] = x_all
    xg_t = np.ascontiguousarray(xg.T.astype(ml_dtypes.bfloat16))

    # weights: wmat[d, k*D + o] = W[o, d, k]
    wmat = W.transpose(1, 2, 0).reshape(D, K * D).astype(ml_dtypes.bfloat16)
    w_in = np.ascontiguousarray(wmat.reshape(2, 128, K * D))

    b2 = np.ascontiguousarray(np.stack([b[:128], b[128:]], axis=1))

    in_maps = []
    for c in range(NCORES):
        xc = np.ascontiguousarray(
            xg_t[:, c * L : c * L + LH].reshape(2, 128, LH)
        )
        in_maps.append({"x": xc, "w": w_in, "b2": b2})
    aux = {"tok_gpos": tok_gpos, "gamma": gamma, "beta": beta}
    return in_maps, aux


def assemble_output(results, aux):
    """Unshard + fold the BatchNorm affine.

    Device sums include the gap columns; subtract their contribution (from
    the very same f32 y values the device summed), reduce across cores,
    then apply y*scale + shift per channel while gathering.
    """
    tok_gpos = aux["tok_gpos"]
    gamma, beta = aux["gamma"], aux["beta"]
    core = tok_gpos // L
    loc = tok_gpos % L

    S = np.zeros(D, dtype=np.float64)
    Q = np.zeros(D, dtype=np.float64)
    for c in range(NCORES):
        st = results[c]["st"].astype(np.float64)
        S += np.concatenate(
            [st[:, 0:NB].sum(axis=1), st[:, NB : 2 * NB].sum(axis=1)]
        )
        Q += np.concatenate(
            [st[:, 2 * NB : 3 * NB].sum(axis=1), st[:, 3 * NB :].sum(axis=1)]
        )
    valid = np.zeros((NCORES, L), dtype=bool)
    valid[core, loc] = True
    for c in range(NCORES):
        yg = results[c]["out"][:, ~valid[c]].astype(np.float64)  # [256, ngap]
        S -= yg.sum(axis=1)
        Q -= (yg * yg).sum(axis=1)

    mean = S / N
    var = Q / N - mean * mean
    scale = gamma.astype(np.float64) / np.sqrt(var + EPS)
    shift = beta.astype(np.float64) - mean * scale
    scale32 = scale.astype(np.float32)
    shift32 = shift.astype(np.float32)

    out = np.empty((N, D), dtype=np.float32)
    for c in range(NCORES):
        sel = core == c
        yc = results[c]["out"][:, loc[sel]].T.astype(np.float32)
        out[sel] = yc * scale32 + shift32
    return out


def kernel(x_all, W, b, gamma, beta, segment_key):
    nc = _get_program()
    in_maps, aux = prepare_inputs(x_all, W, b, gamma, beta, segment_key)
    res = run_bass_kernel_spmd(nc, in_maps, list(range(NCORES)))
    return assemble_output(res.results, aux)
